# revision 16
# baseline (speedup 1.0000x reference)
"""Content-guided attention kernel for Trainium2, 8 NeuronCores SPMD.

Sharding: 8 cores = (batch b in {0,1}) x (query-chunk qc in {0..3}).
Each core computes 1024 query positions of batch b end-to-end:
q/k/vT projections, 8-head attention over all 3072 keys, o-projection,
residual and LayerNorm.  No collectives needed; host splits/concats.

Per-core layout highlights:
 - all matmul operands in fp16 (1 cycle/row on the PE vs 4 for fp32;
   PSUM accumulation stays fp32)
 - scores computed transposed S^T[kpos, qpos] so softmax sum folds into the
   attn@V matmul via a ones-column appended to V^T (no partition reductions)
 - head_dim=32 scores matmuls are packed 4-at-a-time into the PE's 32-row
   groups via tile_position (4x concurrency at K=32)
 - attn@V matmuls col-packed 2-at-a-time (heads at output partitions 0 and
   64 of one PSUM bank) so the M=33 lane waste halves
 - attn@V of score-chunk u issues 2 chunks after its scores, so the exp
   result is always ready when the PE reaches it: the PE stream is
   gap-free, which keeps the HAM clock-gate at full rate (2.4 GHz) instead
   of oscillating back to the cold 1.2 GHz state
 - normalized head outputs stay in the attn@V partition layout; the o-proj
   weight matrix is row-permuted and zero-padded host-side to match, which
   removes the per-head SBUF shuffle DMAs of the numerators
 - o-projection of query block 0 is emitted a few score-chunks into block 1
   so the softmax-normalize tail latency hides behind PE work
 - inputs arrive via few, large DMAs split across both HWDGE queues (SP +
   Activation) because each dma_start costs ~0.65us of trigger time
 - q/k biases folded into the PSUM->SBUF eviction tensor_scalar; v bias via
   tensor_tensor add against a partition-broadcast tile
 - exp split between ScalarE (table exp, fp16 out) and VectorE (Schraudolph
   bit-trick exp emitted as int16 fp16-bitpattern)
 - LayerNorm rstd computed as exp(-0.5*ln(var+eps)) to stay inside the
   single natural_log_exp ACT table set; LN scale/shift run on GpSimd
"""

import numpy as np

C = 256
NH = 8
D = 32
NQ_CORE = 1024
NK = 3072
N_CORES = 8
SCALE = float(D) ** -0.5

# Schraudolph exp constants for an fp16 bit-pattern target:
# bits16 = round(s * SCALE * 1024/ln2 + (15*1024 - 44.56))
_SCHR_A16 = float(np.float32(SCALE * 1024.0 / np.log(2.0)))
_SCHR_B16 = float(np.float32(15.0 * 1024.0 - 44.56))

# exp slots: 3 of every 5 on ScalarE (table exp), 2 of 5 on VectorE
def _use_dve_exp(slot: int) -> bool:
    return slot % 5 >= 3


def _apply_walrus_wait_patch():
    """This walrus build accepts only ONE sync-wait per instruction; split
    extra waits onto single-wait NoOps inserted before the instruction
    (same engine, same block => per-engine program order preserved)."""
    import orjson
    import concourse.bass_utils as bass_utils
    import concourse.bass2jax as bass2jax

    if getattr(bass_utils, "_ant_wait_split_patch", False):
        return
    bass_utils._ant_wait_split_patch = True
    counter = [0]

    def _split_waits(bir_bytes: bytes) -> bytes:
        d = orjson.loads(bir_bytes)
        changed = False

        def process_blocks(blocks):
            nonlocal changed
            for b in blocks:
                insts = b.get("instructions")
                if insts:
                    new = []
                    for ins in insts:
                        si = ins.get("sync_info")
                        waits = si.get("on_wait") if si else None
                        if waits and len(waits) > 1:
                            changed = True
                            for w in waits[:-1]:
                                counter[0] += 1
                                new.append({
                                    "debug": ins.get("debug", 0),
                                    "engine": ins["engine"],
                                    "ins": [],
                                    "outs": [],
                                    "name": f"antwsplit-{counter[0]}",
                                    "opcode": "NoOp",
                                    "sync_info": {"on_wait": [w], "on_update": []},
                                })
                            si["on_wait"] = [waits[-1]]
                        new.append(ins)
                    b["instructions"] = new
                if b.get("blocks"):
                    process_blocks(b["blocks"])

        for f in d.get("functions", []):
            process_blocks(f.get("blocks", []))
        return orjson.dumps(d) if changed else bir_bytes

    orig = bass_utils.compile_bir_kernel

    def compile_bir_kernel(bir, tmpdir, neff_name="file.neff", **kw):
        if isinstance(bir, (bytes, bytearray)):
            bir = _split_waits(bytes(bir))
        elif isinstance(bir, str):
            bir = _split_waits(bir.encode()).decode()
        return orig(bir, tmpdir, neff_name=neff_name, **kw)

    bass_utils.compile_bir_kernel = compile_bir_kernel
    bass2jax.compile_bir_kernel = compile_bir_kernel


def build_program():
    import concourse.bass as bass
    import concourse.tile as tile
    from concourse import mybir

    f32 = mybir.dt.float32
    f16 = mybir.dt.float16
    i16 = mybir.dt.int16
    Alu = mybir.AluOpType
    Act = mybir.ActivationFunctionType

    nc = bass.Bass()

    # x / kv / weights are host-pre-arranged to [128 partitions, chunk, col]
    # so each input lands as ONE DMA with multi-KB contiguous rows (the DMA
    # queues are packet-rate limited: 1KB packets cap ~30 GB/s per queue)
    x_d = nc.dram_tensor("x", [128, 2 * NQ_CORE], f16, kind="ExternalInput")
    kv_d = nc.dram_tensor("kv", [128, 2 * NK], f16, kind="ExternalInput")
    wall_d = nc.dram_tensor("wall", [128, 12 * C], f16, kind="ExternalInput")
    qkb_d = nc.dram_tensor("qkb", [128, 4], f32, kind="ExternalInput")
    rows3_d = nc.dram_tensor("rows3", [1, 3 * C], f32, kind="ExternalInput")
    ob_d = nc.dram_tensor("ob2", [1, C], f16, kind="ExternalInput")
    y_d = nc.dram_tensor("y", [NQ_CORE, C], f32, kind="ExternalOutput")

    def bcast_part(ap, n):
        # partition-stride-0 view: replicate one partition row across n
        # (DRAM sources only; SBUF partition dims need nonzero step)
        return bass.AP(tensor=ap.tensor, offset=ap.offset,
                       ap=[[0, n]] + [list(a) for a in ap.ap[1:]])

    def bcast_sbuf_row(ap, n):
        # SBUF [1, F] row -> [n, F] DMA source: keep the 1-partition dim,
        # replicate via a step-0 free dim (legal for DMA reads)
        return bass.AP(tensor=ap.tensor, offset=ap.offset,
                       ap=[list(ap.ap[0]), [0, n]] + [list(a) for a in ap.ap[1:]])

    from contextlib import ExitStack
    with tile.TileContext(nc) as tc, ExitStack() as ctx:
            consts = ctx.enter_context(tc.tile_pool(name="consts", bufs=1))
            data = ctx.enter_context(tc.tile_pool(name="data", bufs=1))
            acts = ctx.enter_context(tc.tile_pool(name="acts", bufs=1))
            # ---- inputs: few big DMAs, split across the two HWDGE queues.
            # sync queue: weights + x (unblocks q-proj first);
            # scalar queue: kv chunks + small consts.
            w_all = consts.tile([128, 12, C], f16, tag="w_all")
            nc.sync.dma_start(out=w_all, in_=wall_d[:])
            W_Q, W_K, W_V, W_ID, W_OA = 0, 2, 4, 6, 8

            x_sb = data.tile([128, 2, NQ_CORE], f16, tag="x_sb")
            nc.sync.dma_start(out=x_sb, in_=x_d[:])
            kv_sb = data.tile([128, 2, NK], f16, tag="kv_sb")
            nc.scalar.dma_start(out=kv_sb, in_=kv_d[:])
            qkb_col = consts.tile([128, 4], f32, tag="qkb_col")
            nc.scalar.dma_start(out=qkb_col, in_=qkb_d[:])
            rows3_bc = consts.tile([128, 3 * C], f32, tag="rows3_bc")
            nc.scalar.dma_start(out=rows3_bc, in_=bcast_part(rows3_d[:], 128))
            vb_bc = rows3_bc[:, 0:C]
            lnw_bc = rows3_bc[:, C:2 * C]
            lnb_bc = rows3_bc[:, 2 * C:3 * C]
            ob_row = consts.tile([1, C], f16, tag="ob_row")
            nc.scalar.dma_start(out=ob_row, in_=ob_d[:])
            ones_row = consts.tile([1, 512], f16, tag="ones_row")
            nc.vector.memset(ones_row, 1.0)
            eps_col = consts.tile([128, 1], f32, tag="eps_col")
            nc.vector.memset(eps_col, 1e-5)

            q_sb = acts.tile([128, 2, NQ_CORE], f16, tag="q_sb")
            k_sb = acts.tile([128, 2, NK], f16, tag="k_sb")
            vT_aug = acts.tile([128, 24, NH, D + 1], f16, tag="vT_aug")
            nc.vector.memset(vT_aug[:, :, :, D:D + 1], 1.0)
            # onrm[qb][hp]: normalized attn@V numerators in po layout
            # (head A rows 0:32, head B rows 64:96).  Rows 32:64 / 96:128 are
            # dead lanes multiplied by zero-padded owA rows in the o-proj;
            # memset once so stale SBUF NaNs can't propagate through 0*x.
            onrm = [[acts.tile([128, 512], f16, tag=f"onrm_{qb}_{hp}",
                                name=f"onrm_{qb}_{hp}")
                     for hp in range(4)] for qb in range(2)]
            for qb in range(2):
                for hp in range(4):
                    nc.vector.memset(onrm[qb][hp][32:64, :], 0.0)
                    nc.vector.memset(onrm[qb][hp][96:128, :], 0.0)

            # ---- attention + o-proj + LN ----
            # unit = (qb, hg, kc): the 4 heads of group hg against one
            # 128-key chunk.  Scores run 4-way row-group concurrent (four
            # separate PSUM banks); attn@V runs as two col-packed pairs into
            # po_a / po_b.  PSUM budget: sA(2) + sB(2) + poA(2) + poB(2) = 8.
            NU = 2 * 2 * 24
            VDELAY = 2
            with tc.tile_pool(name="sA_ps", bufs=1, space="PSUM") as sA_pool, \
                 tc.tile_pool(name="sB_ps", bufs=1, space="PSUM") as sB_pool, \
                 tc.tile_pool(name="oA_ps", bufs=2, space="PSUM") as oA_pool, \
                 tc.tile_pool(name="oB_ps", bufs=2, space="PSUM") as oB_pool, \
                 tc.tile_pool(name="exps", bufs=3) as exp_pool, \
                 tc.tile_pool(name="tails", bufs=2) as tails, \
                 tc.tile_pool(name="fins", bufs=4) as fins:
                po_tiles = {}
                es_tiles = {}

                def emit_scores(u):
                    qb, hg, kc = u // 48, (u // 24) % 2, u % 24
                    ps_a = sA_pool.tile([128, 2, 512], f32, tag="Sa")
                    ps_b = sB_pool.tile([128, 2, 512], f32, tag="Sb")
                    pss = (ps_a, ps_b)
                    for jj in range(4):
                        pof = 32 * jj
                        nc.tensor.matmul(
                            pss[jj // 2][:, jj % 2, :],
                            lhsT=k_sb[pof:pof + 32, hg, kc * 128:(kc + 1) * 128],
                            rhs=q_sb[pof:pof + 32, hg, qb * 512:(qb + 1) * 512],
                            start=True, stop=True, tile_position=(pof, 0))
                    # pair A exps on ScalarE (table exp), pair B on VectorE
                    # (Schraudolph): engines run in parallel and each softmax
                    # row uses one consistent approximation
                    es_a = exp_pool.tile([128, 2, 512], f16, tag="expA")
                    nc.scalar.activation(es_a, ps_a, Act.Exp, scale=SCALE)
                    es_b_i = exp_pool.tile([128, 2, 512], i16, tag="expB")
                    nc.vector.tensor_scalar(
                        out=es_b_i, in0=ps_b, scalar1=_SCHR_A16,
                        scalar2=_SCHR_B16, op0=Alu.mult, op1=Alu.add)
                    es_tiles[u] = (es_a, es_b_i.bitcast(f16))

                def emit_av(v):
                    # attn@V for unit v (two col-packed head pairs)
                    qb, hg, kc = v // 48, (v // 24) % 2, v % 24
                    hgi = v // 24
                    if kc == 0:
                        po_tiles[hgi] = (
                            oA_pool.tile([128, 512], f32, tag="opoA",
                                         name=f"poA_{hgi}"),
                            oB_pool.tile([128, 512], f32, tag="opoB",
                                         name=f"poB_{hgi}"))
                    po_a, po_b = po_tiles[hgi]
                    es_a, es_b = es_tiles.pop(v)
                    for jj in range(4):
                        po = po_a if jj < 2 else po_b
                        es = es_a if jj < 2 else es_b
                        j = jj % 2
                        nc.tensor.matmul(
                            po[64 * j:64 * j + D + 1, :],
                            lhsT=vT_aug[:, kc, 4 * hg + jj, :],
                            rhs=es[:, j, :],
                            start=(kc == 0), stop=(kc == 23),
                            tile_position=(0, 64 * j))

                def emit_tail(hgi):
                    # numerators stay put in po layout; only the four
                    # denominator rows (32 / 96 of each po) leave PSUM.
                    # Spread the 2048 denominators over 64 partitions via
                    # DMA, recip, broadcast back per head.
                    qb, hg = hgi // 2, hgi % 2
                    po_a, po_b = po_tiles.pop(hgi)
                    raw_a = tails.tile([128, 512], f32, tag="rawA")
                    nc.vector.tensor_copy(raw_a, po_a)
                    raw_b = tails.tile([128, 512], f32, tag="rawB")
                    nc.vector.tensor_copy(raw_b, po_b)
                    dp = tails.tile([64, 32], f32, tag="dp")
                    nc.sync.dma_start(out=dp[0:32, :], in_=raw_a[D:64 + D + 1:64, :])
                    nc.scalar.dma_start(out=dp[32:64, :], in_=raw_b[D:64 + D + 1:64, :])
                    rp = tails.tile([64, 32], f16, tag="rp")
                    with nc.allow_low_precision(reason="softmax denom recip in fp16"):
                        nc.vector.reciprocal(rp, dp)
                    rec = tails.tile([1, 4, 512], f16, tag="rec")
                    nc.sync.dma_start(
                        out=rec.rearrange("p a q -> p (a q)"), in_=rp)
                    rbc_a = tails.tile([128, 512], f16, tag="rbcA")
                    rbc_b = tails.tile([128, 512], f16, tag="rbcB")
                    nc.sync.dma_start(out=rbc_a[0:32, :],
                                      in_=bcast_sbuf_row(rec[0:1, 0, :], 32))
                    nc.scalar.dma_start(out=rbc_a[64:96, :],
                                        in_=bcast_sbuf_row(rec[0:1, 1, :], 32))
                    nc.sync.dma_start(out=rbc_b[0:32, :],
                                      in_=bcast_sbuf_row(rec[0:1, 2, :], 32))
                    nc.scalar.dma_start(out=rbc_b[64:96, :],
                                        in_=bcast_sbuf_row(rec[0:1, 3, :], 32))
                    for pi, (po, rbc) in enumerate(((po_a, rbc_a), (po_b, rbc_b))):
                        hp = 2 * hg + pi
                        for j in range(2):
                            nc.vector.tensor_mul(
                                onrm[qb][hp][64 * j:64 * j + 32, :],
                                po[64 * j:64 * j + 32, :],
                                rbc[64 * j:64 * j + 32, :])

                def emit_oproj(qb):
                    # o-projection + residual + bias + LayerNorm per 128
                    # queries.  pso pairs borrow the po pools -- the only
                    # PSUM banks that free up while the attention stream of
                    # the next block is still running.
                    # qb0 mid-stream: borrow the po pools (score pools are
                    # hot every unit).  qb1 at stream end: borrow the score
                    # pools, which free right after the last exp -- the
                    # o-proj then overlaps the final softmax tail chain.
                    pA, tA = (oA_pool, "opoA") if qb == 0 else (sA_pool, "Sa")
                    pB, tB = (oB_pool, "opoB") if qb == 0 else (sB_pool, "Sb")
                    psoA = pA.tile([128, 2, C], f32, tag=tA, name=f"psoA_{qb}")
                    psoB = pB.tile([128, 2, C], f32, tag=tB, name=f"psoB_{qb}")
                    for qc2 in range(4):
                        qoff = qb * 512 + qc2 * 128
                        pso = (psoA, psoB)[qc2 // 2][:, qc2 % 2, :]
                        for hp in range(4):
                            nc.tensor.matmul(
                                pso, lhsT=onrm[qb][hp][:, qc2 * 128:(qc2 + 1) * 128],
                                rhs=w_all[:, W_OA + hp, :],
                                start=(hp == 0), stop=False)
                        for cc in range(2):
                            nc.tensor.matmul(
                                pso, lhsT=x_sb[:, cc, qoff:qoff + 128],
                                rhs=w_all[:, W_ID + cc, :], start=False, stop=False)
                        nc.tensor.matmul(pso, lhsT=ones_row[0:1, 0:128], rhs=ob_row[:],
                                         start=False, stop=True)
                        stats = fins.tile([128, 6], f32, tag="stats")
                        nc.vector.bn_stats(stats, pso)
                        mv = fins.tile([128, 2], f32, tag="mv")
                        nc.vector.bn_aggr(mv, stats)
                        # rstd = exp(-0.5*ln(var+eps)): stays in the same ACT
                        # table set as the softmax exp (no table reload)
                        lnv = fins.tile([128, 1], f32, tag="lnv")
                        nc.scalar.activation(lnv, mv[:, 1:2], Act.Ln, bias=eps_col[:, 0:1])
                        rstd = fins.tile([128, 1], f32, tag="rstd")
                        nc.scalar.activation(rstd, lnv, Act.Exp, scale=-0.5)
                        t1 = fins.tile([128, C], f32, tag="t1")
                        nc.vector.tensor_scalar(
                            out=t1, in0=pso, scalar1=mv[:, 0:1], scalar2=rstd,
                            op0=Alu.subtract, op1=Alu.mult)
                        # scale/shift on GpSimd mid-stream, DVE at the end
                        eng = nc.gpsimd if qb == 0 else nc.vector
                        t2 = fins.tile([128, C], f32, tag="t2")
                        eng.tensor_mul(t2, t1, lnw_bc)
                        t3 = fins.tile([128, C], f32, tag="t3")
                        eng.tensor_add(t3, t2, lnb_bc)
                        nc.sync.dma_start(out=y_d[qoff:qoff + 128, :], in_=t3)

                next_u = [0]

                def pump(upto):
                    while next_u[0] < upto:
                        u = next_u[0]
                        next_u[0] += 1
                        if u < NU:
                            emit_scores(u)
                        if u >= VDELAY:
                            v = u - VDELAY
                            emit_av(v)
                            if v % 24 == 23:
                                emit_tail(v // 24)
                        if u == 48 + 4:
                            emit_oproj(0)

                # ---- projections, interleaved with the attention stream.
                # Proj PSUM tiles borrow the score pools (no spare banks);
                # after each kv chunk's k/v projections, the units that
                # chunk unblocks are emitted so the PE never drains while
                # later chunks are still in flight over DMA.
                def proj_tile(i):
                    pool, tag = ((sA_pool, "Sa"), (sB_pool, "Sb"))[i % 2]
                    return pool.tile([128, 2, 512], f32, tag=tag,
                                     name=f"projps_{i}")

                pcnt = [0]
                for mc in range(2):
                    for nb in range(2):
                        ps = proj_tile(pcnt[0])[:, 0, :]
                        pcnt[0] += 1
                        for kc2 in range(2):
                            nc.tensor.matmul(
                                ps, lhsT=w_all[:, W_Q + kc2, mc * 128:(mc + 1) * 128],
                                rhs=x_sb[:, kc2, nb * 512:(nb + 1) * 512],
                                start=(kc2 == 0), stop=(kc2 == 1))
                        nc.vector.tensor_scalar_add(
                            out=q_sb[:, mc, nb * 512:(nb + 1) * 512], in0=ps,
                            scalar1=qkb_col[:, mc:mc + 1])
                for nb in range(6):
                    for mc in range(2):
                        ps = proj_tile(pcnt[0])[:, 0, :]
                        pcnt[0] += 1
                        for kc2 in range(2):
                            nc.tensor.matmul(
                                ps, lhsT=w_all[:, W_K + kc2, mc * 128:(mc + 1) * 128],
                                rhs=kv_sb[:, kc2, nb * 512:(nb + 1) * 512],
                                start=(kc2 == 0), stop=(kc2 == 1))
                        nc.vector.tensor_scalar_add(
                            out=k_sb[:, mc, nb * 512:(nb + 1) * 512], in0=ps,
                            scalar1=qkb_col[:, 2 + mc:3 + mc])
                    for nn in range(4 * nb, 4 * nb + 4):
                        ps = proj_tile(pcnt[0])[:, 0, 0:C]
                        pcnt[0] += 1
                        for kc2 in range(2):
                            nc.tensor.matmul(
                                ps, lhsT=kv_sb[:, kc2, nn * 128:(nn + 1) * 128],
                                rhs=w_all[:, W_V + kc2, :], start=(kc2 == 0),
                                stop=(kc2 == 1))
                        nc.vector.tensor_add(
                            vT_aug[:, nn, :, 0:D],
                            ps.rearrange("p (h e) -> p h e", h=NH),
                            vb_bc.rearrange("p (h e) -> p h e", h=NH))
                    # chunk nb unblocks units kc <= 4*nb+3 of (qb0, hg0)
                    if nb < 5:
                        pump(4 * (nb + 1))
                pump(NU + VDELAY)
                emit_oproj(1)
    return nc


_CACHE = {}


def _get_program():
    if "nc" not in _CACHE:
        _apply_walrus_wait_patch()
        _CACHE["nc"] = build_program()
    return _CACHE["nc"]


def _make_in_maps(inputs):
    s3 = np.asarray(inputs["s3"], dtype=np.float32)
    s4 = np.asarray(inputs["s4"], dtype=np.float32)
    s5 = np.asarray(inputs["s5"], dtype=np.float32)
    wts = {}
    for nm in ("qw", "kw", "vw"):
        wts[nm + "T"] = np.asarray(inputs[nm], dtype=np.float32).T.astype(np.float16)
    # o-proj weights permuted + zero-padded to the attn@V PSUM layout:
    # chunk hp rows = [head 2hp (32) | zeros (32) | head 2hp+1 (32) | zeros]
    owT = np.asarray(inputs["ow"], dtype=np.float32).T.astype(np.float16)
    owA = np.zeros((4, 128, C), dtype=np.float16)
    for hp in range(4):
        owA[hp, 0:32] = owT[(2 * hp) * 32:(2 * hp) * 32 + 32]
        owA[hp, 64:96] = owT[(2 * hp + 1) * 32:(2 * hp + 1) * 32 + 32]
    ident = np.eye(C, dtype=np.float16)
    wall = np.ascontiguousarray(np.concatenate(
        [wts["qwT"], wts["kwT"], wts["vwT"], ident, owA.reshape(512, C)], axis=0))
    qkb = np.ascontiguousarray(np.stack(
        [np.asarray(inputs["qb"], np.float32).reshape(2, 128)[0],
         np.asarray(inputs["qb"], np.float32).reshape(2, 128)[1],
         np.asarray(inputs["kb"], np.float32).reshape(2, 128)[0],
         np.asarray(inputs["kb"], np.float32).reshape(2, 128)[1]], axis=1))
    rows3 = np.ascontiguousarray(np.concatenate(
        [np.asarray(inputs["vb"], np.float32).reshape(1, C),
         np.asarray(inputs["ln_w"], np.float32).reshape(1, C),
         np.asarray(inputs["ln_b"], np.float32).reshape(1, C)], axis=1))
    ob_r = np.asarray(inputs["ob"], dtype=np.float32).reshape(1, C).astype(np.float16)
    # pre-arrange to the on-device [128, chunk, col] layout (row a*128+p of
    # the logical [C, N] tensor lands at partition p, chunk a)
    def prearrange(t):
        a = t.shape[0] // 128
        return np.ascontiguousarray(
            t.reshape(a, 128, -1).transpose(1, 0, 2).reshape(128, -1))

    wall_r = prearrange(wall)
    in_maps = []
    for core in range(N_CORES):
        b, qc = core // 4, core % 4
        x = prearrange(
            s3[b].reshape(C, -1)[:, qc * NQ_CORE:(qc + 1) * NQ_CORE]
            .astype(np.float16))
        kv = prearrange(np.concatenate(
            [s4[b].reshape(C, -1), s5[b].reshape(C, -1)], axis=1)
            .astype(np.float16))
        in_maps.append({
            "x": x, "kv": kv, "wall": wall_r,
            "qkb": qkb, "rows3": rows3, "ob2": ob_r,
        })
    return in_maps


def _assemble(results, like):
    B, _, H, W = 2, C, 64, 64
    out = np.empty((B, C, H * W), dtype=np.float32)
    for core in range(N_CORES):
        b, qc = core // 4, core % 4
        out[b, :, qc * NQ_CORE:(qc + 1) * NQ_CORE] = results[core]["y"].T
    return out.reshape(B, C, H, W)


def kernel(**inputs):
    from concourse import bass2jax
    nc = _get_program()
    in_maps = _make_in_maps(inputs)
    results = bass2jax.run_bass_via_pjrt(nc, in_maps, n_cores=N_CORES)
    return _assemble(results, inputs["s3"])


# revision 17
# speedup vs baseline: 1.1855x; 1.1855x over previous
"""Content-guided attention kernel for Trainium2, 8 NeuronCores SPMD.

Sharding: 8 cores = (batch b in {0,1}) x (query-chunk qc in {0..3}).
Each core computes 1024 query positions of batch b end-to-end:
q/k/vT projections, 8-head attention over all 3072 keys, o-projection,
residual and LayerNorm.  No collectives needed; host splits/concats.

Per-core layout highlights:
 - all matmul operands in fp16 (1 cycle/row on the PE vs 4 for fp32;
   PSUM accumulation stays fp32)
 - scores computed transposed S^T[kpos, qpos] so softmax sum folds into the
   attn@V matmul via a ones-column appended to V^T (no partition reductions)
 - head_dim=32 scores matmuls are packed 4-at-a-time into the PE's 32-row
   groups via tile_position (4x concurrency at K=32)
 - attn@V matmuls col-packed 2-at-a-time (heads at output partitions 0 and
   64 of one PSUM bank) so the M=33 lane waste halves
 - attn@V of score-chunk u issues 2 chunks after its scores, so the exp
   result is always ready when the PE reaches it: the PE stream is
   gap-free, which keeps the HAM clock-gate at full rate (2.4 GHz) instead
   of oscillating back to the cold 1.2 GHz state
 - normalized head outputs stay in the attn@V partition layout; the o-proj
   weight matrix is row-permuted and zero-padded host-side to match, which
   removes the per-head SBUF shuffle DMAs of the numerators
 - o-projection of query block 0 is emitted a few score-chunks into block 1
   so the softmax-normalize tail latency hides behind PE work
 - inputs arrive via few, large DMAs split across both HWDGE queues (SP +
   Activation) because each dma_start costs ~0.65us of trigger time
 - q/k biases folded into the PSUM->SBUF eviction tensor_scalar; v bias via
   tensor_tensor add against a partition-broadcast tile
 - exp split between ScalarE (table exp, fp16 out) and VectorE (Schraudolph
   bit-trick exp emitted as int16 fp16-bitpattern)
 - LayerNorm rstd computed as exp(-0.5*ln(var+eps)) to stay inside the
   single natural_log_exp ACT table set; LN scale/shift run on GpSimd
"""

import numpy as np

C = 256
NH = 8
D = 32
NQ_CORE = 1024
NK = 3072
N_CORES = 8
SCALE = float(D) ** -0.5

# Schraudolph exp constants for an fp16 bit-pattern target:
# bits16 = round(s * SCALE * 1024/ln2 + (15*1024 - 44.56))
_SCHR_A16 = float(np.float32(SCALE * 1024.0 / np.log(2.0)))
_SCHR_B16 = float(np.float32(15.0 * 1024.0 - 44.56))

# exp slots: 3 of every 5 on ScalarE (table exp), 2 of 5 on VectorE
def _use_dve_exp(slot: int) -> bool:
    return slot % 5 >= 3


def _apply_walrus_wait_patch():
    """This walrus build accepts only ONE sync-wait per instruction; split
    extra waits onto single-wait NoOps inserted before the instruction
    (same engine, same block => per-engine program order preserved)."""
    import orjson
    import concourse.bass_utils as bass_utils
    import concourse.bass2jax as bass2jax

    if getattr(bass_utils, "_ant_wait_split_patch", False):
        return
    bass_utils._ant_wait_split_patch = True
    counter = [0]

    def _split_waits(bir_bytes: bytes) -> bytes:
        d = orjson.loads(bir_bytes)
        changed = False

        def process_blocks(blocks):
            nonlocal changed
            for b in blocks:
                insts = b.get("instructions")
                if insts:
                    new = []
                    for ins in insts:
                        si = ins.get("sync_info")
                        waits = si.get("on_wait") if si else None
                        if waits and len(waits) > 1:
                            changed = True
                            for w in waits[:-1]:
                                counter[0] += 1
                                new.append({
                                    "debug": ins.get("debug", 0),
                                    "engine": ins["engine"],
                                    "ins": [],
                                    "outs": [],
                                    "name": f"antwsplit-{counter[0]}",
                                    "opcode": "NoOp",
                                    "sync_info": {"on_wait": [w], "on_update": []},
                                })
                            si["on_wait"] = [waits[-1]]
                        new.append(ins)
                    b["instructions"] = new
                if b.get("blocks"):
                    process_blocks(b["blocks"])

        for f in d.get("functions", []):
            process_blocks(f.get("blocks", []))
        return orjson.dumps(d) if changed else bir_bytes

    orig = bass_utils.compile_bir_kernel

    def compile_bir_kernel(bir, tmpdir, neff_name="file.neff", **kw):
        if isinstance(bir, (bytes, bytearray)):
            bir = _split_waits(bytes(bir))
        elif isinstance(bir, str):
            bir = _split_waits(bir.encode()).decode()
        return orig(bir, tmpdir, neff_name=neff_name, **kw)

    bass_utils.compile_bir_kernel = compile_bir_kernel
    bass2jax.compile_bir_kernel = compile_bir_kernel


def build_program():
    import concourse.bass as bass
    import concourse.tile as tile
    from concourse import mybir

    f32 = mybir.dt.float32
    f16 = mybir.dt.float16
    i16 = mybir.dt.int16
    Alu = mybir.AluOpType
    Act = mybir.ActivationFunctionType

    nc = bass.Bass()

    # x / kv / weights are host-pre-arranged to [128 partitions, chunk, col]
    # so each input lands as ONE DMA with multi-KB contiguous rows (the DMA
    # queues are packet-rate limited: 1KB packets cap ~30 GB/s per queue)
    x_d = nc.dram_tensor("x", [128, 2 * NQ_CORE], f16, kind="ExternalInput")
    kv_d = nc.dram_tensor("kv", [128, 2 * NK], f16, kind="ExternalInput")
    wall_d = nc.dram_tensor("wall", [128, 12 * C], f16, kind="ExternalInput")
    qkb_d = nc.dram_tensor("qkb", [128, 4], f32, kind="ExternalInput")
    rows3_d = nc.dram_tensor("rows3", [1, 3 * C], f32, kind="ExternalInput")
    ob_d = nc.dram_tensor("ob2", [1, C], f16, kind="ExternalInput")
    y_d = nc.dram_tensor("y", [NQ_CORE, C], f32, kind="ExternalOutput")

    def bcast_part(ap, n):
        # partition-stride-0 view: replicate one partition row across n
        # (DRAM sources only; SBUF partition dims need nonzero step)
        return bass.AP(tensor=ap.tensor, offset=ap.offset,
                       ap=[[0, n]] + [list(a) for a in ap.ap[1:]])

    def bcast_sbuf_row(ap, n):
        # SBUF [1, F] row -> [n, F] DMA source: keep the 1-partition dim,
        # replicate via a step-0 free dim (legal for DMA reads)
        return bass.AP(tensor=ap.tensor, offset=ap.offset,
                       ap=[list(ap.ap[0]), [0, n]] + [list(a) for a in ap.ap[1:]])

    from contextlib import ExitStack
    with tile.TileContext(nc) as tc, ExitStack() as ctx:
            consts = ctx.enter_context(tc.tile_pool(name="consts", bufs=1))
            data = ctx.enter_context(tc.tile_pool(name="data", bufs=1))
            acts = ctx.enter_context(tc.tile_pool(name="acts", bufs=1))
            # ---- inputs: few big DMAs, split across the two HWDGE queues.
            # sync queue: weights + x (unblocks q-proj first);
            # scalar queue: kv chunks + small consts.
            w_all = consts.tile([128, 12, C], f16, tag="w_all")
            nc.sync.dma_start(out=w_all, in_=wall_d[:])
            W_Q, W_K, W_V, W_ID, W_OA = 0, 2, 4, 6, 8

            x_sb = data.tile([128, 2, NQ_CORE], f16, tag="x_sb")
            nc.sync.dma_start(out=x_sb, in_=x_d[:])
            kv_sb = data.tile([128, 2, NK], f16, tag="kv_sb")
            nc.scalar.dma_start(out=kv_sb, in_=kv_d[:])
            qkb_col = consts.tile([128, 4], f32, tag="qkb_col")
            nc.scalar.dma_start(out=qkb_col, in_=qkb_d[:])
            rows3_bc = consts.tile([128, 3 * C], f32, tag="rows3_bc")
            nc.scalar.dma_start(out=rows3_bc, in_=bcast_part(rows3_d[:], 128))
            vb_bc = rows3_bc[:, 0:C]
            lnw_bc = rows3_bc[:, C:2 * C]
            lnb_bc = rows3_bc[:, 2 * C:3 * C]
            ob_row = consts.tile([1, C], f16, tag="ob_row")
            nc.scalar.dma_start(out=ob_row, in_=ob_d[:])
            ones_row = consts.tile([1, 512], f16, tag="ones_row")
            nc.vector.memset(ones_row, 1.0)
            eps_col = consts.tile([128, 1], f32, tag="eps_col")
            nc.vector.memset(eps_col, 1e-5)

            q_sb = acts.tile([128, 2, NQ_CORE], f16, tag="q_sb")
            k_sb = acts.tile([128, 2, NK], f16, tag="k_sb")
            vT_aug = acts.tile([128, 24, NH, D + 1], f16, tag="vT_aug")
            nc.vector.memset(vT_aug[:, :, :, D:D + 1], 1.0)
            # onrm[qb][hp]: normalized attn@V numerators in po layout
            # (head A rows 0:32, head B rows 64:96).  Rows 32:64 / 96:128 are
            # dead lanes multiplied by zero-padded owA rows in the o-proj;
            # memset once so stale SBUF NaNs can't propagate through 0*x.
            onrm = [[acts.tile([128, 512], f16, tag=f"onrm_{qb}_{hp}",
                                name=f"onrm_{qb}_{hp}")
                     for hp in range(4)] for qb in range(2)]
            for qb in range(2):
                for hp in range(4):
                    nc.vector.memset(onrm[qb][hp][32:64, :], 0.0)
                    nc.vector.memset(onrm[qb][hp][96:128, :], 0.0)

            # ---- attention + o-proj + LN ----
            # unit = (qb, hg, kc): the 4 heads of group hg against one
            # 128-key chunk.  Scores run 4-way row-group concurrent (four
            # separate PSUM banks); attn@V runs as two col-packed pairs into
            # po_a / po_b.  PSUM budget: sA(2) + sB(2) + poA(2) + poB(2) = 8.
            NU = 2 * 2 * 24
            VDELAY = 2
            with tc.tile_pool(name="sA_ps", bufs=1, space="PSUM") as sA_pool, \
                 tc.tile_pool(name="sB_ps", bufs=1, space="PSUM") as sB_pool, \
                 tc.tile_pool(name="oA_ps", bufs=2, space="PSUM") as oA_pool, \
                 tc.tile_pool(name="oB_ps", bufs=2, space="PSUM") as oB_pool, \
                 tc.tile_pool(name="exps", bufs=3) as exp_pool, \
                 tc.tile_pool(name="tails", bufs=2) as tails, \
                 tc.tile_pool(name="fins", bufs=4) as fins:
                po_tiles = {}
                es_tiles = {}

                def emit_scores(u):
                    qb, hg, kc = u // 48, (u // 24) % 2, u % 24
                    ps_a = sA_pool.tile([128, 2, 512], f32, tag="Sa")
                    ps_b = sB_pool.tile([128, 2, 512], f32, tag="Sb")
                    pss = (ps_a, ps_b)
                    for jj in range(4):
                        pof = 32 * jj
                        nc.tensor.matmul(
                            pss[jj // 2][:, jj % 2, :],
                            lhsT=k_sb[pof:pof + 32, hg, kc * 128:(kc + 1) * 128],
                            rhs=q_sb[pof:pof + 32, hg, qb * 512:(qb + 1) * 512],
                            start=True, stop=True, tile_position=(pof, 0))
                    # pair A exps on ScalarE (table exp), pair B on VectorE
                    # (Schraudolph): engines run in parallel and each softmax
                    # row uses one consistent approximation
                    es_a = exp_pool.tile([128, 2, 512], f16, tag="expA")
                    nc.scalar.activation(es_a, ps_a, Act.Exp, scale=SCALE)
                    es_b_i = exp_pool.tile([128, 2, 512], i16, tag="expB")
                    nc.vector.tensor_scalar(
                        out=es_b_i, in0=ps_b, scalar1=_SCHR_A16,
                        scalar2=_SCHR_B16, op0=Alu.mult, op1=Alu.add)
                    es_tiles[u] = (es_a, es_b_i.bitcast(f16))

                def emit_av(v):
                    # attn@V for unit v (two col-packed head pairs)
                    qb, hg, kc = v // 48, (v // 24) % 2, v % 24
                    hgi = v // 24
                    if kc == 0:
                        po_tiles[hgi] = (
                            oA_pool.tile([128, 512], f32, tag="opoA",
                                         name=f"poA_{hgi}"),
                            oB_pool.tile([128, 512], f32, tag="opoB",
                                         name=f"poB_{hgi}"))
                    po_a, po_b = po_tiles[hgi]
                    es_a, es_b = es_tiles.pop(v)
                    for jj in range(4):
                        po = po_a if jj < 2 else po_b
                        es = es_a if jj < 2 else es_b
                        j = jj % 2
                        nc.tensor.matmul(
                            po[64 * j:64 * j + D + 1, :],
                            lhsT=vT_aug[:, kc, 4 * hg + jj, :],
                            rhs=es[:, j, :],
                            start=(kc == 0), stop=(kc == 23),
                            tile_position=(0, 64 * j))

                def emit_tail(hgi):
                    # numerators stay put in po layout; only the four
                    # denominator rows (32 / 96 of each po) leave PSUM.
                    # Spread the 2048 denominators over 64 partitions via
                    # DMA, recip, broadcast back per head.
                    qb, hg = hgi // 2, hgi % 2
                    po_a, po_b = po_tiles.pop(hgi)
                    raw_a = tails.tile([128, 512], f32, tag="rawA")
                    nc.vector.tensor_copy(raw_a, po_a)
                    raw_b = tails.tile([128, 512], f32, tag="rawB")
                    nc.vector.tensor_copy(raw_b, po_b)
                    dp = tails.tile([64, 32], f32, tag="dp")
                    nc.sync.dma_start(out=dp[0:32, :], in_=raw_a[D:64 + D + 1:64, :])
                    nc.scalar.dma_start(out=dp[32:64, :], in_=raw_b[D:64 + D + 1:64, :])
                    rp = tails.tile([64, 32], f16, tag="rp")
                    with nc.allow_low_precision(reason="softmax denom recip in fp16"):
                        nc.vector.reciprocal(rp, dp)
                    rec = tails.tile([1, 4, 512], f16, tag="rec")
                    nc.sync.dma_start(
                        out=rec.rearrange("p a q -> p (a q)"), in_=rp)
                    rbc_a = tails.tile([128, 512], f16, tag="rbcA")
                    rbc_b = tails.tile([128, 512], f16, tag="rbcB")
                    nc.sync.dma_start(out=rbc_a[0:32, :],
                                      in_=bcast_sbuf_row(rec[0:1, 0, :], 32))
                    nc.scalar.dma_start(out=rbc_a[64:96, :],
                                        in_=bcast_sbuf_row(rec[0:1, 1, :], 32))
                    nc.sync.dma_start(out=rbc_b[0:32, :],
                                      in_=bcast_sbuf_row(rec[0:1, 2, :], 32))
                    nc.scalar.dma_start(out=rbc_b[64:96, :],
                                        in_=bcast_sbuf_row(rec[0:1, 3, :], 32))
                    for pi, (po, rbc) in enumerate(((po_a, rbc_a), (po_b, rbc_b))):
                        hp = 2 * hg + pi
                        for j in range(2):
                            nc.vector.tensor_mul(
                                onrm[qb][hp][64 * j:64 * j + 32, :],
                                po[64 * j:64 * j + 32, :],
                                rbc[64 * j:64 * j + 32, :])

                def emit_oproj(qb):
                    # o-projection + residual + bias + LayerNorm per 128
                    # queries.  pso pairs borrow the po pools -- the only
                    # PSUM banks that free up while the attention stream of
                    # the next block is still running.
                    psoA = oA_pool.tile([128, 2, C], f32, tag="opoA",
                                        name=f"psoA_{qb}")
                    psoB = oB_pool.tile([128, 2, C], f32, tag="opoB",
                                        name=f"psoB_{qb}")
                    for qc2 in range(4):
                        qoff = qb * 512 + qc2 * 128
                        pso = (psoA, psoB)[qc2 // 2][:, qc2 % 2, :]
                        for hp in range(4):
                            nc.tensor.matmul(
                                pso, lhsT=onrm[qb][hp][:, qc2 * 128:(qc2 + 1) * 128],
                                rhs=w_all[:, W_OA + hp, :],
                                start=(hp == 0), stop=False)
                        for cc in range(2):
                            nc.tensor.matmul(
                                pso, lhsT=x_sb[:, cc, qoff:qoff + 128],
                                rhs=w_all[:, W_ID + cc, :], start=False, stop=False)
                        nc.tensor.matmul(pso, lhsT=ones_row[0:1, 0:128], rhs=ob_row[:],
                                         start=False, stop=True)
                        stats = fins.tile([128, 6], f32, tag="stats")
                        nc.vector.bn_stats(stats, pso)
                        mv = fins.tile([128, 2], f32, tag="mv")
                        nc.vector.bn_aggr(mv, stats)
                        # rstd = exp(-0.5*ln(var+eps)): stays in the same ACT
                        # table set as the softmax exp (no table reload)
                        lnv = fins.tile([128, 1], f32, tag="lnv")
                        nc.scalar.activation(lnv, mv[:, 1:2], Act.Ln, bias=eps_col[:, 0:1])
                        rstd = fins.tile([128, 1], f32, tag="rstd")
                        nc.scalar.activation(rstd, lnv, Act.Exp, scale=-0.5)
                        t1 = fins.tile([128, C], f32, tag="t1")
                        nc.vector.tensor_scalar(
                            out=t1, in0=pso, scalar1=mv[:, 0:1], scalar2=rstd,
                            op0=Alu.subtract, op1=Alu.mult)
                        # scale/shift on GpSimd mid-stream, DVE at the end
                        eng = nc.gpsimd if qb == 0 else nc.vector
                        t2 = fins.tile([128, C], f32, tag="t2")
                        eng.tensor_mul(t2, t1, lnw_bc)
                        t3 = fins.tile([128, C], f32, tag="t3")
                        eng.tensor_add(t3, t2, lnb_bc)
                        nc.sync.dma_start(out=y_d[qoff:qoff + 128, :], in_=t3)

                next_u = [0]

                def pump(upto):
                    while next_u[0] < upto:
                        u = next_u[0]
                        next_u[0] += 1
                        if u < NU:
                            emit_scores(u)
                        if u >= VDELAY:
                            v = u - VDELAY
                            emit_av(v)
                            if v % 24 == 23:
                                emit_tail(v // 24)
                        if u == 48 + 4:
                            emit_oproj(0)

                # ---- projections, interleaved with the attention stream.
                # Proj PSUM tiles borrow the score pools (no spare banks);
                # after each kv chunk's k/v projections, the units that
                # chunk unblocks are emitted so the PE never drains while
                # later chunks are still in flight over DMA.
                def proj_tile(i):
                    pool, tag = ((sA_pool, "Sa"), (sB_pool, "Sb"))[i % 2]
                    return pool.tile([128, 2, 512], f32, tag=tag,
                                     name=f"projps_{i}")

                pcnt = [0]
                for mc in range(2):
                    for nb in range(2):
                        ps = proj_tile(pcnt[0])[:, 0, :]
                        pcnt[0] += 1
                        for kc2 in range(2):
                            nc.tensor.matmul(
                                ps, lhsT=w_all[:, W_Q + kc2, mc * 128:(mc + 1) * 128],
                                rhs=x_sb[:, kc2, nb * 512:(nb + 1) * 512],
                                start=(kc2 == 0), stop=(kc2 == 1))
                        nc.vector.tensor_scalar_add(
                            out=q_sb[:, mc, nb * 512:(nb + 1) * 512], in0=ps,
                            scalar1=qkb_col[:, mc:mc + 1])
                for nb in range(6):
                    for mc in range(2):
                        ps = proj_tile(pcnt[0])[:, 0, :]
                        pcnt[0] += 1
                        for kc2 in range(2):
                            nc.tensor.matmul(
                                ps, lhsT=w_all[:, W_K + kc2, mc * 128:(mc + 1) * 128],
                                rhs=kv_sb[:, kc2, nb * 512:(nb + 1) * 512],
                                start=(kc2 == 0), stop=(kc2 == 1))
                        nc.vector.tensor_scalar_add(
                            out=k_sb[:, mc, nb * 512:(nb + 1) * 512], in0=ps,
                            scalar1=qkb_col[:, 2 + mc:3 + mc])
                    for nn in range(4 * nb, 4 * nb + 4):
                        ps = proj_tile(pcnt[0])[:, 0, 0:C]
                        pcnt[0] += 1
                        for kc2 in range(2):
                            nc.tensor.matmul(
                                ps, lhsT=kv_sb[:, kc2, nn * 128:(nn + 1) * 128],
                                rhs=w_all[:, W_V + kc2, :], start=(kc2 == 0),
                                stop=(kc2 == 1))
                        nc.vector.tensor_add(
                            vT_aug[:, nn, :, 0:D],
                            ps.rearrange("p (h e) -> p h e", h=NH),
                            vb_bc.rearrange("p (h e) -> p h e", h=NH))
                    # chunk nb unblocks units kc <= 4*nb+3 of (qb0, hg0)
                    if nb < 5:
                        pump(4 * (nb + 1))
                pump(NU + VDELAY)
                emit_oproj(1)
    return nc


_CACHE = {}


def _get_program():
    if "nc" not in _CACHE:
        _apply_walrus_wait_patch()
        _CACHE["nc"] = build_program()
    return _CACHE["nc"]


def _make_in_maps(inputs):
    s3 = np.asarray(inputs["s3"], dtype=np.float32)
    s4 = np.asarray(inputs["s4"], dtype=np.float32)
    s5 = np.asarray(inputs["s5"], dtype=np.float32)
    wts = {}
    for nm in ("qw", "kw", "vw"):
        wts[nm + "T"] = np.asarray(inputs[nm], dtype=np.float32).T.astype(np.float16)
    # o-proj weights permuted + zero-padded to the attn@V PSUM layout:
    # chunk hp rows = [head 2hp (32) | zeros (32) | head 2hp+1 (32) | zeros]
    owT = np.asarray(inputs["ow"], dtype=np.float32).T.astype(np.float16)
    owA = np.zeros((4, 128, C), dtype=np.float16)
    for hp in range(4):
        owA[hp, 0:32] = owT[(2 * hp) * 32:(2 * hp) * 32 + 32]
        owA[hp, 64:96] = owT[(2 * hp + 1) * 32:(2 * hp + 1) * 32 + 32]
    ident = np.eye(C, dtype=np.float16)
    wall = np.ascontiguousarray(np.concatenate(
        [wts["qwT"], wts["kwT"], wts["vwT"], ident, owA.reshape(512, C)], axis=0))
    qkb = np.ascontiguousarray(np.stack(
        [np.asarray(inputs["qb"], np.float32).reshape(2, 128)[0],
         np.asarray(inputs["qb"], np.float32).reshape(2, 128)[1],
         np.asarray(inputs["kb"], np.float32).reshape(2, 128)[0],
         np.asarray(inputs["kb"], np.float32).reshape(2, 128)[1]], axis=1))
    rows3 = np.ascontiguousarray(np.concatenate(
        [np.asarray(inputs["vb"], np.float32).reshape(1, C),
         np.asarray(inputs["ln_w"], np.float32).reshape(1, C),
         np.asarray(inputs["ln_b"], np.float32).reshape(1, C)], axis=1))
    ob_r = np.asarray(inputs["ob"], dtype=np.float32).reshape(1, C).astype(np.float16)
    # pre-arrange to the on-device [128, chunk, col] layout (row a*128+p of
    # the logical [C, N] tensor lands at partition p, chunk a)
    def prearrange(t):
        a = t.shape[0] // 128
        return np.ascontiguousarray(
            t.reshape(a, 128, -1).transpose(1, 0, 2).reshape(128, -1))

    wall_r = prearrange(wall)
    in_maps = []
    for core in range(N_CORES):
        b, qc = core // 4, core % 4
        x = prearrange(
            s3[b].reshape(C, -1)[:, qc * NQ_CORE:(qc + 1) * NQ_CORE]
            .astype(np.float16))
        kv = prearrange(np.concatenate(
            [s4[b].reshape(C, -1), s5[b].reshape(C, -1)], axis=1)
            .astype(np.float16))
        in_maps.append({
            "x": x, "kv": kv, "wall": wall_r,
            "qkb": qkb, "rows3": rows3, "ob2": ob_r,
        })
    return in_maps


def _assemble(results, like):
    B, _, H, W = 2, C, 64, 64
    out = np.empty((B, C, H * W), dtype=np.float32)
    for core in range(N_CORES):
        b, qc = core // 4, core % 4
        out[b, :, qc * NQ_CORE:(qc + 1) * NQ_CORE] = results[core]["y"].T
    return out.reshape(B, C, H, W)


def kernel(**inputs):
    from concourse import bass2jax
    nc = _get_program()
    in_maps = _make_in_maps(inputs)
    results = bass2jax.run_bass_via_pjrt(nc, in_maps, n_cores=N_CORES)
    return _assemble(results, inputs["s3"])


# revision 18
# speedup vs baseline: 1.2391x; 1.0452x over previous
"""Content-guided attention kernel for Trainium2, 8 NeuronCores SPMD.

Sharding: 8 cores = (batch b in {0,1}) x (query-chunk qc in {0..3}).
Each core computes 1024 query positions of batch b end-to-end:
q/k/vT projections, 8-head attention over all 3072 keys, o-projection,
residual and LayerNorm.  No collectives needed; host splits/concats.

Per-core layout highlights:
 - all matmul operands in fp16 (1 cycle/row on the PE vs 4 for fp32;
   PSUM accumulation stays fp32)
 - scores computed transposed S^T[kpos, qpos] so softmax sum folds into the
   attn@V matmul via a ones-column appended to V^T (no partition reductions)
 - head_dim=32 scores matmuls are packed 4-at-a-time into the PE's 32-row
   groups via tile_position (4x concurrency at K=32)
 - attn@V matmuls col-packed 2-at-a-time (heads at output partitions 0 and
   64 of one PSUM bank) so the M=33 lane waste halves
 - attn@V of score-chunk u issues 2 chunks after its scores, so the exp
   result is always ready when the PE reaches it: the PE stream is
   gap-free, which keeps the HAM clock-gate at full rate (2.4 GHz) instead
   of oscillating back to the cold 1.2 GHz state
 - normalized head outputs stay in the attn@V partition layout; the o-proj
   weight matrix is row-permuted and zero-padded host-side to match, which
   removes the per-head SBUF shuffle DMAs of the numerators
 - o-projection of query block 0 is emitted a few score-chunks into block 1
   so the softmax-normalize tail latency hides behind PE work
 - inputs arrive via few, large DMAs split across both HWDGE queues (SP +
   Activation) because each dma_start costs ~0.65us of trigger time
 - q/k biases folded into the PSUM->SBUF eviction tensor_scalar; v bias via
   tensor_tensor add against a partition-broadcast tile
 - exp split between ScalarE (table exp, fp16 out) and VectorE (Schraudolph
   bit-trick exp emitted as int16 fp16-bitpattern)
 - LayerNorm rstd computed as exp(-0.5*ln(var+eps)) to stay inside the
   single natural_log_exp ACT table set; LN scale/shift run on GpSimd
"""

import numpy as np

C = 256
NH = 8
D = 32
NQ_CORE = 1024
NK = 3072
N_CORES = 8
SCALE = float(D) ** -0.5

# Schraudolph exp constants for an fp16 bit-pattern target:
# bits16 = round(s * SCALE * 1024/ln2 + (15*1024 - 44.56))
_SCHR_A16 = float(np.float32(SCALE * 1024.0 / np.log(2.0)))
_SCHR_B16 = float(np.float32(15.0 * 1024.0 - 44.56))

# exp slots: 3 of every 5 on ScalarE (table exp), 2 of 5 on VectorE
def _use_dve_exp(slot: int) -> bool:
    return slot % 5 >= 3


def _apply_walrus_wait_patch():
    """This walrus build accepts only ONE sync-wait per instruction; split
    extra waits onto single-wait NoOps inserted before the instruction
    (same engine, same block => per-engine program order preserved)."""
    import orjson
    import concourse.bass_utils as bass_utils
    import concourse.bass2jax as bass2jax

    if getattr(bass_utils, "_ant_wait_split_patch", False):
        return
    bass_utils._ant_wait_split_patch = True
    counter = [0]

    def _split_waits(bir_bytes: bytes) -> bytes:
        d = orjson.loads(bir_bytes)
        changed = False

        def process_blocks(blocks):
            nonlocal changed
            for b in blocks:
                insts = b.get("instructions")
                if insts:
                    new = []
                    for ins in insts:
                        si = ins.get("sync_info")
                        waits = si.get("on_wait") if si else None
                        if waits and len(waits) > 1:
                            changed = True
                            for w in waits[:-1]:
                                counter[0] += 1
                                new.append({
                                    "debug": ins.get("debug", 0),
                                    "engine": ins["engine"],
                                    "ins": [],
                                    "outs": [],
                                    "name": f"antwsplit-{counter[0]}",
                                    "opcode": "NoOp",
                                    "sync_info": {"on_wait": [w], "on_update": []},
                                })
                            si["on_wait"] = [waits[-1]]
                        new.append(ins)
                    b["instructions"] = new
                if b.get("blocks"):
                    process_blocks(b["blocks"])

        for f in d.get("functions", []):
            process_blocks(f.get("blocks", []))
        return orjson.dumps(d) if changed else bir_bytes

    orig = bass_utils.compile_bir_kernel

    def compile_bir_kernel(bir, tmpdir, neff_name="file.neff", **kw):
        if isinstance(bir, (bytes, bytearray)):
            bir = _split_waits(bytes(bir))
        elif isinstance(bir, str):
            bir = _split_waits(bir.encode()).decode()
        return orig(bir, tmpdir, neff_name=neff_name, **kw)

    bass_utils.compile_bir_kernel = compile_bir_kernel
    bass2jax.compile_bir_kernel = compile_bir_kernel


def build_program():
    import concourse.bass as bass
    import concourse.tile as tile
    from concourse import mybir

    f32 = mybir.dt.float32
    f16 = mybir.dt.float16
    i16 = mybir.dt.int16
    Alu = mybir.AluOpType
    Act = mybir.ActivationFunctionType

    nc = bass.Bass()

    # x / kv / weights are host-pre-arranged to [128 partitions, chunk, col]
    # so each input lands as ONE DMA with multi-KB contiguous rows (the DMA
    # queues are packet-rate limited: 1KB packets cap ~30 GB/s per queue)
    x_d = nc.dram_tensor("x", [128, 2 * NQ_CORE], f16, kind="ExternalInput")
    kv_d = nc.dram_tensor("kv", [128, 2 * NK], f16, kind="ExternalInput")
    wall_d = nc.dram_tensor("wall", [128, 12 * C], f16, kind="ExternalInput")
    qkb_d = nc.dram_tensor("qkb", [128, 4], f32, kind="ExternalInput")
    rows3_d = nc.dram_tensor("rows3", [1, 3 * C], f32, kind="ExternalInput")
    ob_d = nc.dram_tensor("ob2", [1, C], f16, kind="ExternalInput")
    y_d = nc.dram_tensor("y", [NQ_CORE, C], f32, kind="ExternalOutput")

    def bcast_part(ap, n):
        # partition-stride-0 view: replicate one partition row across n
        # (DRAM sources only; SBUF partition dims need nonzero step)
        return bass.AP(tensor=ap.tensor, offset=ap.offset,
                       ap=[[0, n]] + [list(a) for a in ap.ap[1:]])

    def bcast_sbuf_row(ap, n):
        # SBUF [1, F] row -> [n, F] DMA source: keep the 1-partition dim,
        # replicate via a step-0 free dim (legal for DMA reads)
        return bass.AP(tensor=ap.tensor, offset=ap.offset,
                       ap=[list(ap.ap[0]), [0, n]] + [list(a) for a in ap.ap[1:]])

    from contextlib import ExitStack
    with tile.TileContext(nc) as tc, ExitStack() as ctx:
            consts = ctx.enter_context(tc.tile_pool(name="consts", bufs=1))
            data = ctx.enter_context(tc.tile_pool(name="data", bufs=1))
            acts = ctx.enter_context(tc.tile_pool(name="acts", bufs=1))
            # ---- inputs: few big DMAs, split across the two HWDGE queues.
            # sync queue: weights + x (unblocks q-proj first);
            # scalar queue: kv chunks + small consts.
            w_all = consts.tile([128, 12, C], f16, tag="w_all")
            nc.sync.dma_start(out=w_all, in_=wall_d[:])
            W_Q, W_K, W_V, W_ID, W_OA = 0, 2, 4, 6, 8

            x_sb = data.tile([128, 2, NQ_CORE], f16, tag="x_sb")
            nc.sync.dma_start(out=x_sb, in_=x_d[:])
            kv_sb = data.tile([128, 2, NK], f16, tag="kv_sb")
            nc.scalar.dma_start(out=kv_sb, in_=kv_d[:])
            qkb_col = consts.tile([128, 4], f32, tag="qkb_col")
            nc.scalar.dma_start(out=qkb_col, in_=qkb_d[:])
            rows3_bc = consts.tile([128, 3 * C], f32, tag="rows3_bc")
            nc.scalar.dma_start(out=rows3_bc, in_=bcast_part(rows3_d[:], 128))
            vb_bc = rows3_bc[:, 0:C]
            lnw_bc = rows3_bc[:, C:2 * C]
            lnb_bc = rows3_bc[:, 2 * C:3 * C]
            ob_row = consts.tile([1, C], f16, tag="ob_row")
            nc.scalar.dma_start(out=ob_row, in_=ob_d[:])
            ones_row = consts.tile([1, 512], f16, tag="ones_row")
            nc.vector.memset(ones_row, 1.0)
            eps_col = consts.tile([128, 1], f32, tag="eps_col")
            nc.vector.memset(eps_col, 1e-5)

            q_sb = acts.tile([128, 2, NQ_CORE], f16, tag="q_sb")
            k_sb = acts.tile([128, 2, NK], f16, tag="k_sb")
            vT_aug = acts.tile([128, 24, NH, D + 1], f16, tag="vT_aug")
            nc.vector.memset(vT_aug[:, :, :, D:D + 1], 1.0)
            # onrm[qb][hp]: normalized attn@V numerators in po layout
            # (head A rows 0:32, head B rows 64:96).  Rows 32:64 / 96:128 are
            # dead lanes multiplied by zero-padded owA rows in the o-proj;
            # memset once so stale SBUF NaNs can't propagate through 0*x.
            onrm = [[acts.tile([128, 512], f16, tag=f"onrm_{qb}_{hp}",
                                name=f"onrm_{qb}_{hp}")
                     for hp in range(4)] for qb in range(2)]
            for qb in range(2):
                for hp in range(4):
                    nc.vector.memset(onrm[qb][hp][32:64, :], 0.0)
                    nc.vector.memset(onrm[qb][hp][96:128, :], 0.0)

            # ---- attention + o-proj + LN ----
            # unit = (qb, hg, kc): the 4 heads of group hg against one
            # 128-key chunk.  Scores run 4-way row-group concurrent (four
            # separate PSUM banks); attn@V runs as two col-packed pairs into
            # po_a / po_b.  PSUM budget: sA(2) + sB(2) + poA(2) + poB(2) = 8.
            NU = 2 * 2 * 24
            VDELAY = 2
            with tc.tile_pool(name="sA_ps", bufs=1, space="PSUM") as sA_pool, \
                 tc.tile_pool(name="sB_ps", bufs=1, space="PSUM") as sB_pool, \
                 tc.tile_pool(name="oA_ps", bufs=2, space="PSUM") as oA_pool, \
                 tc.tile_pool(name="oB_ps", bufs=2, space="PSUM") as oB_pool, \
                 tc.tile_pool(name="exps", bufs=3) as exp_pool, \
                 tc.tile_pool(name="tails", bufs=3) as tails, \
                 tc.tile_pool(name="fins", bufs=6) as fins:
                po_tiles = {}
                es_tiles = {}

                def emit_scores(u):
                    qb, hg, kc = u // 48, (u // 24) % 2, u % 24
                    ps_a = sA_pool.tile([128, 2, 512], f32, tag="Sa")
                    ps_b = sB_pool.tile([128, 2, 512], f32, tag="Sb")
                    pss = (ps_a, ps_b)
                    for jj in range(4):
                        pof = 32 * jj
                        nc.tensor.matmul(
                            pss[jj // 2][:, jj % 2, :],
                            lhsT=k_sb[pof:pof + 32, hg, kc * 128:(kc + 1) * 128],
                            rhs=q_sb[pof:pof + 32, hg, qb * 512:(qb + 1) * 512],
                            start=True, stop=True, tile_position=(pof, 0))
                    # pair A exps on ScalarE (table exp), pair B on VectorE
                    # (Schraudolph): engines run in parallel and each softmax
                    # row uses one consistent approximation
                    es_a = exp_pool.tile([128, 2, 512], f16, tag="expA")
                    nc.scalar.activation(es_a, ps_a, Act.Exp, scale=SCALE)
                    es_b_i = exp_pool.tile([128, 2, 512], i16, tag="expB")
                    nc.vector.tensor_scalar(
                        out=es_b_i, in0=ps_b, scalar1=_SCHR_A16,
                        scalar2=_SCHR_B16, op0=Alu.mult, op1=Alu.add)
                    es_tiles[u] = (es_a, es_b_i.bitcast(f16))

                def emit_av(v):
                    # attn@V for unit v (two col-packed head pairs)
                    qb, hg, kc = v // 48, (v // 24) % 2, v % 24
                    hgi = v // 24
                    if kc == 0:
                        po_tiles[hgi] = (
                            oA_pool.tile([128, 512], f32, tag="opoA",
                                         name=f"poA_{hgi}"),
                            oB_pool.tile([128, 512], f32, tag="opoB",
                                         name=f"poB_{hgi}"))
                    po_a, po_b = po_tiles[hgi]
                    es_a, es_b = es_tiles.pop(v)
                    for jj in range(4):
                        po = po_a if jj < 2 else po_b
                        es = es_a if jj < 2 else es_b
                        j = jj % 2
                        nc.tensor.matmul(
                            po[64 * j:64 * j + D + 1, :],
                            lhsT=vT_aug[:, kc, 4 * hg + jj, :],
                            rhs=es[:, j, :],
                            start=(kc == 0), stop=(kc == 23),
                            tile_position=(0, 64 * j))

                def emit_tail(hgi):
                    # numerators stay put in po layout; only the four
                    # denominator rows (32 / 96 of each po) leave PSUM.
                    # Spread the 2048 denominators over 64 partitions via
                    # DMA, recip, broadcast back per head.
                    qb, hg = hgi // 2, hgi % 2
                    po_a, po_b = po_tiles.pop(hgi)
                    raw_a = tails.tile([128, 512], f32, tag="rawA")
                    nc.scalar.copy(raw_a, po_a)
                    raw_b = tails.tile([128, 512], f32, tag="rawB")
                    nc.scalar.copy(raw_b, po_b)
                    dp = tails.tile([64, 32], f32, tag="dp")
                    nc.sync.dma_start(out=dp[0:32, :], in_=raw_a[D:64 + D + 1:64, :])
                    nc.scalar.dma_start(out=dp[32:64, :], in_=raw_b[D:64 + D + 1:64, :])
                    rp = tails.tile([64, 32], f16, tag="rp")
                    with nc.allow_low_precision(reason="softmax denom recip in fp16"):
                        nc.vector.reciprocal(rp, dp)
                    rec = tails.tile([1, 4, 512], f16, tag="rec")
                    nc.sync.dma_start(
                        out=rec.rearrange("p a q -> p (a q)"), in_=rp)
                    rbc_a = tails.tile([128, 512], f16, tag="rbcA")
                    rbc_b = tails.tile([128, 512], f16, tag="rbcB")
                    nc.sync.dma_start(out=rbc_a[0:32, :],
                                      in_=bcast_sbuf_row(rec[0:1, 0, :], 32))
                    nc.scalar.dma_start(out=rbc_a[64:96, :],
                                        in_=bcast_sbuf_row(rec[0:1, 1, :], 32))
                    nc.sync.dma_start(out=rbc_b[0:32, :],
                                      in_=bcast_sbuf_row(rec[0:1, 2, :], 32))
                    nc.scalar.dma_start(out=rbc_b[64:96, :],
                                        in_=bcast_sbuf_row(rec[0:1, 3, :], 32))
                    for pi, (po, rbc) in enumerate(((po_a, rbc_a), (po_b, rbc_b))):
                        hp = 2 * hg + pi
                        for j in range(2):
                            nc.vector.tensor_mul(
                                onrm[qb][hp][64 * j:64 * j + 32, :],
                                po[64 * j:64 * j + 32, :],
                                rbc[64 * j:64 * j + 32, :])

                def emit_oproj(qb):
                    # o-projection + residual + bias + LayerNorm per 128
                    # queries.  pso pairs borrow the po pools -- the only
                    # PSUM banks that free up while the attention stream of
                    # the next block is still running.
                    psoA = oA_pool.tile([128, 2, C], f32, tag="opoA",
                                        name=f"psoA_{qb}")
                    psoB = oB_pool.tile([128, 2, C], f32, tag="opoB",
                                        name=f"psoB_{qb}")
                    for qc2 in range(4):
                        qoff = qb * 512 + qc2 * 128
                        pso = (psoA, psoB)[qc2 // 2][:, qc2 % 2, :]
                        for hp in range(4):
                            nc.tensor.matmul(
                                pso, lhsT=onrm[qb][hp][:, qc2 * 128:(qc2 + 1) * 128],
                                rhs=w_all[:, W_OA + hp, :],
                                start=(hp == 0), stop=False)
                        for cc in range(2):
                            nc.tensor.matmul(
                                pso, lhsT=x_sb[:, cc, qoff:qoff + 128],
                                rhs=w_all[:, W_ID + cc, :], start=False, stop=False)
                        nc.tensor.matmul(pso, lhsT=ones_row[0:1, 0:128], rhs=ob_row[:],
                                         start=False, stop=True)
                        stats = fins.tile([128, 6], f32, tag="stats")
                        nc.vector.bn_stats(stats, pso)
                        mv = fins.tile([128, 2], f32, tag="mv")
                        nc.vector.bn_aggr(mv, stats)
                        # rstd = exp(-0.5*ln(var+eps)): stays in the same ACT
                        # table set as the softmax exp (no table reload)
                        lnv = fins.tile([128, 1], f32, tag="lnv")
                        nc.scalar.activation(lnv, mv[:, 1:2], Act.Ln, bias=eps_col[:, 0:1])
                        rstd = fins.tile([128, 1], f32, tag="rstd")
                        nc.scalar.activation(rstd, lnv, Act.Exp, scale=-0.5)
                        t1 = fins.tile([128, C], f32, tag="t1")
                        nc.vector.tensor_scalar(
                            out=t1, in0=pso, scalar1=mv[:, 0:1], scalar2=rstd,
                            op0=Alu.subtract, op1=Alu.mult)
                        # scale/shift on GpSimd mid-stream, DVE at the end
                        eng = nc.gpsimd if qb == 0 else nc.vector
                        t2 = fins.tile([128, C], f32, tag="t2")
                        eng.tensor_mul(t2, t1, lnw_bc)
                        t3 = fins.tile([128, C], f32, tag="t3")
                        eng.tensor_add(t3, t2, lnb_bc)
                        nc.sync.dma_start(out=y_d[qoff:qoff + 128, :], in_=t3)

                next_u = [0]

                def pump(upto):
                    while next_u[0] < upto:
                        u = next_u[0]
                        next_u[0] += 1
                        if u < NU:
                            emit_scores(u)
                        if u >= VDELAY:
                            v = u - VDELAY
                            emit_av(v)
                            if v % 24 == 23:
                                emit_tail(v // 24)
                        if u == 48 + 4:
                            emit_oproj(0)

                # ---- projections, interleaved with the attention stream.
                # Proj PSUM tiles borrow the score pools (no spare banks);
                # after each kv chunk's k/v projections, the units that
                # chunk unblocks are emitted so the PE never drains while
                # later chunks are still in flight over DMA.
                def proj_tile(i):
                    pool, tag = ((sA_pool, "Sa"), (sB_pool, "Sb"))[i % 2]
                    return pool.tile([128, 2, 512], f32, tag=tag,
                                     name=f"projps_{i}")

                pcnt = [0]
                for mc in range(2):
                    for nb in range(2):
                        ps = proj_tile(pcnt[0])[:, 0, :]
                        pcnt[0] += 1
                        for kc2 in range(2):
                            nc.tensor.matmul(
                                ps, lhsT=w_all[:, W_Q + kc2, mc * 128:(mc + 1) * 128],
                                rhs=x_sb[:, kc2, nb * 512:(nb + 1) * 512],
                                start=(kc2 == 0), stop=(kc2 == 1))
                        nc.vector.tensor_scalar_add(
                            out=q_sb[:, mc, nb * 512:(nb + 1) * 512], in0=ps,
                            scalar1=qkb_col[:, mc:mc + 1])
                for nb in range(6):
                    for mc in range(2):
                        ps = proj_tile(pcnt[0])[:, 0, :]
                        pcnt[0] += 1
                        for kc2 in range(2):
                            nc.tensor.matmul(
                                ps, lhsT=w_all[:, W_K + kc2, mc * 128:(mc + 1) * 128],
                                rhs=kv_sb[:, kc2, nb * 512:(nb + 1) * 512],
                                start=(kc2 == 0), stop=(kc2 == 1))
                        nc.vector.tensor_scalar_add(
                            out=k_sb[:, mc, nb * 512:(nb + 1) * 512], in0=ps,
                            scalar1=qkb_col[:, 2 + mc:3 + mc])
                    for nn in range(4 * nb, 4 * nb + 4):
                        ps = proj_tile(pcnt[0])[:, 0, 0:C]
                        pcnt[0] += 1
                        for kc2 in range(2):
                            nc.tensor.matmul(
                                ps, lhsT=kv_sb[:, kc2, nn * 128:(nn + 1) * 128],
                                rhs=w_all[:, W_V + kc2, :], start=(kc2 == 0),
                                stop=(kc2 == 1))
                        nc.vector.tensor_add(
                            vT_aug[:, nn, :, 0:D],
                            ps.rearrange("p (h e) -> p h e", h=NH),
                            vb_bc.rearrange("p (h e) -> p h e", h=NH))
                    # chunk nb unblocks units kc <= 4*nb+3 of (qb0, hg0)
                    if nb < 5:
                        pump(4 * (nb + 1))
                pump(NU + VDELAY)
                emit_oproj(1)
    return nc


_CACHE = {}


def _get_program():
    if "nc" not in _CACHE:
        _apply_walrus_wait_patch()
        _CACHE["nc"] = build_program()
    return _CACHE["nc"]


def _make_in_maps(inputs):
    s3 = np.asarray(inputs["s3"], dtype=np.float32)
    s4 = np.asarray(inputs["s4"], dtype=np.float32)
    s5 = np.asarray(inputs["s5"], dtype=np.float32)
    wts = {}
    for nm in ("qw", "kw", "vw"):
        wts[nm + "T"] = np.asarray(inputs[nm], dtype=np.float32).T.astype(np.float16)
    # o-proj weights permuted + zero-padded to the attn@V PSUM layout:
    # chunk hp rows = [head 2hp (32) | zeros (32) | head 2hp+1 (32) | zeros]
    owT = np.asarray(inputs["ow"], dtype=np.float32).T.astype(np.float16)
    owA = np.zeros((4, 128, C), dtype=np.float16)
    for hp in range(4):
        owA[hp, 0:32] = owT[(2 * hp) * 32:(2 * hp) * 32 + 32]
        owA[hp, 64:96] = owT[(2 * hp + 1) * 32:(2 * hp + 1) * 32 + 32]
    ident = np.eye(C, dtype=np.float16)
    wall = np.ascontiguousarray(np.concatenate(
        [wts["qwT"], wts["kwT"], wts["vwT"], ident, owA.reshape(512, C)], axis=0))
    qkb = np.ascontiguousarray(np.stack(
        [np.asarray(inputs["qb"], np.float32).reshape(2, 128)[0],
         np.asarray(inputs["qb"], np.float32).reshape(2, 128)[1],
         np.asarray(inputs["kb"], np.float32).reshape(2, 128)[0],
         np.asarray(inputs["kb"], np.float32).reshape(2, 128)[1]], axis=1))
    rows3 = np.ascontiguousarray(np.concatenate(
        [np.asarray(inputs["vb"], np.float32).reshape(1, C),
         np.asarray(inputs["ln_w"], np.float32).reshape(1, C),
         np.asarray(inputs["ln_b"], np.float32).reshape(1, C)], axis=1))
    ob_r = np.asarray(inputs["ob"], dtype=np.float32).reshape(1, C).astype(np.float16)
    # pre-arrange to the on-device [128, chunk, col] layout (row a*128+p of
    # the logical [C, N] tensor lands at partition p, chunk a)
    def prearrange(t):
        a = t.shape[0] // 128
        return np.ascontiguousarray(
            t.reshape(a, 128, -1).transpose(1, 0, 2).reshape(128, -1))

    wall_r = prearrange(wall)
    in_maps = []
    for core in range(N_CORES):
        b, qc = core // 4, core % 4
        x = prearrange(
            s3[b].reshape(C, -1)[:, qc * NQ_CORE:(qc + 1) * NQ_CORE]
            .astype(np.float16))
        kv = prearrange(np.concatenate(
            [s4[b].reshape(C, -1), s5[b].reshape(C, -1)], axis=1)
            .astype(np.float16))
        in_maps.append({
            "x": x, "kv": kv, "wall": wall_r,
            "qkb": qkb, "rows3": rows3, "ob2": ob_r,
        })
    return in_maps


def _assemble(results, like):
    B, _, H, W = 2, C, 64, 64
    out = np.empty((B, C, H * W), dtype=np.float32)
    for core in range(N_CORES):
        b, qc = core // 4, core % 4
        out[b, :, qc * NQ_CORE:(qc + 1) * NQ_CORE] = results[core]["y"].T
    return out.reshape(B, C, H, W)


def kernel(**inputs):
    from concourse import bass2jax
    nc = _get_program()
    in_maps = _make_in_maps(inputs)
    results = bass2jax.run_bass_via_pjrt(nc, in_maps, n_cores=N_CORES)
    return _assemble(results, inputs["s3"])
